# revision 72
# baseline (speedup 1.0000x reference)
"""GQA attention block (qk-rmsnorm + RoPE + causal GQA attention + out-proj),
tensor-parallel over 8 NeuronCores: 2-way data parallel (batch) x 4-way head
parallel (8 q heads / 2 kv heads per core). All-reduce of out-proj partials is
done on host (sum of 4 partials per batch).

Per-core layouts (device):
  phase 1: q/k/v projections with x^T chunks stationary on PE -> [T,d] rows;
           qk-rmsnorm + RoPE in row layout; PE-transpose q,k to [d,T].
           Early phase-2 blocks are pumped into the phase-1 tail so the exp
           stream (ACT) overlaps the QKV matmuls.
  phase 2: pair-jobs: S^T for BOTH kv heads concurrently on PE (K=64 row
           tiles at partitions 0/64, 4ns stagger) into one [128,1024] psum
           (head B packed against head A so the diag jobs' exp range is
           contiguous and junk-free); causal diag via -30000 mask-add
           matmuls; exp on ACT (scale=1/8, ln(1/64) bias folded) -> P~ f16;
           PV via [V|1] stationary -> out^T + rowsum in one psum per head;
           rowsum -> PE row-broadcast -> DVE reciprocal -> normalize.
  phase 3: out-proj in single-psum-bank passes, interleaved into phase 2;
           f16 output partials (summed in f32 on host).
"""
import sys
import numpy as np

sys.path.insert(0, "/opt/trn_rl_repo")

import concourse.bass as bass  # noqa: E402
import concourse.bacc as bacc  # noqa: E402
import concourse.mybir as mybir  # noqa: E402
import concourse.tile as tile  # noqa: E402
from concourse import masks  # noqa: E402
from concourse.bass_utils import run_bass_kernel_spmd  # noqa: E402

f32 = mybir.dt.float32
f16 = mybir.dt.float16
FT = mybir.ActivationFunctionType
AX = mybir.AxisListType

P = 128
T = 2048
H = 2048
D = 64
NQ = 8          # q heads per core
DQ = NQ * D     # 512
NTT = T // P    # 16 T tiles
NHC = H // P    # 16 hidden chunks
NBLK = 4        # T_q blocks of 512
BLK = 512
EPS = 1e-5
LN64 = -4.1588830833596715  # ln(1/64): scales exp to keep 1/rowsum in f16 normal range
DEPTH = 7       # S->PV pipeline lag (jobs)

_CACHE = {}


def _build_program():
    nc = bacc.Bacc("TRN2", target_bir_lowering=False, debug=False, num_devices=8)

    xT_d = nc.dram_tensor("xT", [H, T], f16, kind="ExternalInput")
    wqkv_d = nc.dram_tensor("wqkv", [H, 768], f16, kind="ExternalInput")
    wo_d = nc.dram_tensor("wo", [DQ, H], f16, kind="ExternalInput")
    ropeq_d = nc.dram_tensor("ropeq", [T, 128], f16, kind="ExternalInput")
    ropek_d = nc.dram_tensor("ropek", [T, 128], f16, kind="ExternalInput")
    mtab_d = nc.dram_tensor("mtab", [P, 256], f16, kind="ExternalInput")
    out_d = nc.dram_tensor("out", [T, H], f16, kind="ExternalOutput")

    with tile.TileContext(nc) as tc:
        with (
            tc.tile_pool(name="persist", bufs=1) as pp,
            tc.tile_pool(name="work", bufs=2) as wp,
            tc.tile_pool(name="ptp", bufs=DEPTH + 2) as ptp,
            tc.tile_pool(name="obp", bufs=12) as obp,
            tc.tile_pool(name="psum", bufs=2, space="PSUM") as ps,
            tc.tile_pool(name="psum_o", bufs=2, space="PSUM") as pso,
        ):
            # ---------- persistent loads ----------
            # Ordered + spread over 3 DGE queues (sync/scalar/gpsimd) so the
            # first QKV matmul's inputs land within ~4us.
            # wqkv arrives host-rearranged as [p*16+hc, c] so per-partition
            # DMA lines are long contiguous runs
            wqkv_all = pp.tile([P, NHC * 768], f16, tag="wqkv")
            wq4 = wqkv_all[:].rearrange("p (h c) -> p h c", c=768)
            wqs = wqkv_d[:].rearrange("(p h) c -> p h c", h=NHC)
            # xT arrives host-pretransposed as [tt*128+p, hc*128+c] so each
            # (partition, tile) DMA line is 4KB contiguous (128 descriptors
            # per tile instead of 2048 x 256B)
            xt_all = pp.tile([P, NTT * H], f16, tag="xt")
            xt4 = xt_all[:].rearrange("p (t h c) -> p t h c", h=NHC, c=P)
            xts = xT_d[:].rearrange("(t p) (h c) -> p t h c", p=P, c=P)
            # sync queue carries wqkv in consumption order while the gpsimd
            # queue carries the first tile's x quarters + ropes in parallel
            # (two DGE streams from t=0 instead of one serialized queue)
            ropeq_sb = pp.tile([P, 16 * 128], f16, tag="ropeq")
            ropek_sb = pp.tile([P, 16 * 128], f16, tag="ropek")
            mtab = pp.tile([P, 256], f16, tag="mtab")
            nc.sync.dma_start(wq4[:, 0:2], wqs[:, 0:2])
            nc.gpsimd.dma_start(xt4[:, 0:1, 0:4], xts[:, 0:1, 0:4])
            nc.sync.dma_start(wq4[:, 2:4], wqs[:, 2:4])
            nc.gpsimd.dma_start(xt4[:, 0:1, 4:8], xts[:, 0:1, 4:8])
            nc.sync.dma_start(wq4[:, 4:7], wqs[:, 4:7])
            nc.gpsimd.dma_start(xt4[:, 0:1, 8:12], xts[:, 0:1, 8:12])
            nc.sync.dma_start(wq4[:, 7:11], wqs[:, 7:11])
            nc.gpsimd.dma_start(xt4[:, 0:1, 12:16], xts[:, 0:1, 12:16])
            nc.sync.dma_start(wq4[:, 11:16], wqs[:, 11:16])
            # rope tables host-rearranged to [p*16+tt, c] for 4KB DMA lines
            nc.gpsimd.dma_start(ropeq_sb[:].rearrange("p (t c) -> p t c", c=128),
                                ropeq_d[:].rearrange("(p t) c -> p t c", t=NTT))
            nc.gpsimd.dma_start(ropek_sb[:].rearrange("p (t c) -> p t c", c=128),
                                ropek_d[:].rearrange("(p t) c -> p t c", t=NTT))
            nc.gpsimd.dma_start(mtab[:], mtab_d[:])
            for t0 in range(1, NTT):
                nc.sync.dma_start(xt4[:, t0:t0 + 1], xts[:, t0:t0 + 1])
            # wo_all's 4MB DMA is triggered from inside the tile loop (at
            # tt==2) so it doesn't steal DMA bandwidth from the critical
            # wqkv/xt0 stream during startup
            wo_all = pp.tile([P, 4 * H], f16, tag="wo")
            negI = mtab[:, 0:128]     # -30000 on diagonal
            ustr = mtab[:, 128:256]   # 1 where k > g (strict lower)
            ident = pp.tile([P, P], f16, tag="ident")
            masks.make_identity(nc, ident[:])
            ones = pp.tile([P, 65], f16, tag="ones")
            nc.gpsimd.memset(ones[:], 1.0)
            lnb = pp.tile([P, 1], f32, tag="lnb")
            nc.gpsimd.memset(lnb[:], LN64)
            epsb = pp.tile([P, 1], f32, tag="epsb")
            nc.gpsimd.memset(epsb[:], EPS)
            # every activation in this kernel (Square/Ln/Exp/Copy) lives in
            # the natural_log_exp_and_others table set: load it once up front
            nc.scalar.add_instruction(mybir.InstLoadActFuncSet(
                name="manual_actload", ins=[], outs=[], act_func_set_id=6))

            qT = pp.tile([P, 4 * T], f16, tag="qT")    # chunk c at cols [c*T,(c+1)*T)
            kT = pp.tile([P, T], f16, tag="kT")        # kv0 rows 0:64, kv1 rows 64:128
            # vsb ones-cols memset on the DVE so the gpsimd queue reaches
            # the first tiles' rope chains sooner
            vsb = []
            for tt in range(NTT):
                vt = pp.tile([P, 130], f16, tag=f"v{tt}")
                nc.vector.memset(vt[:, 64:65], 1.0)     # ones col for kv0
                nc.vector.memset(vt[:, 129:130], 1.0)   # ones col for kv1
                vsb.append(vt)

            qT3 = qT[:].rearrange("p (c t) -> p c t", t=T)

            # ---------- phase 2/3 machinery (driven by pump()) ----------
            class Unit:      # one (block i, q-dim chunk c) pair-unit, both kv heads
                pass

            def emit_S(u, j):
                # S^T chunk j for BOTH kv heads: two K=64 matmuls in disjoint
                # PE row groups (partitions 0/64) -> run concurrently.
                # Head A's valid cols land at [rel:512], head B's at
                # [512:1024-rel] (packed against A) so exp covers the
                # contiguous junk-free range [rel:1024-rel].
                i = u.i
                rel = max(0, (j - 4 * i) * P)
                diag = (j >= 4 * i)
                pa2 = ps.tile([P, 1024], f32, tag="a", name="pa2")
                for s in range(2):
                    nc.tensor.matmul(
                        pa2[:, s * 512 + (rel if s == 0 else 0):
                            (s + 1) * 512 - (0 if s == 0 else rel)],
                        kT[s * D:(s + 1) * D, j * P:(j + 1) * P],
                        qT3[s * D:(s + 1) * D, u.c, i * BLK + rel:(i + 1) * BLK],
                        start=True, stop=not diag, skip_group_check=True)
                if diag:
                    # add -30000 above the diagonal of the 128-block (after
                    # both S halves so the K=64 pair overlaps on the PE);
                    # both heads' diag blocks start at their first valid col
                    for s in range(2):
                        off = rel if s == 0 else 512
                        nc.tensor.matmul(
                            pa2[:, off:off + P],
                            negI, ustr, start=False, stop=True,
                            skip_group_check=True)
                pt = ptp.tile([P, 1024], f16, tag="pt", name="pt")
                nc.scalar.activation(pt[:, rel:1024 - rel], pa2[:, rel:1024 - rel],
                                     FT.Exp, scale=0.125, bias=lnb[:])
                u.pts[j] = (pt, rel)

            def emit_PV(u, j):
                pt, rel = u.pts.pop(j)
                last = 4 * (u.i + 1) - 1
                for s in range(2):
                    nc.tensor.matmul(
                        u.po[s][:, rel:BLK],
                        vsb[j][:, s * 65:s * 65 + 65],
                        pt[:, s * 512:(s + 1) * 512 - rel] if s
                        else pt[:, rel:512],
                        start=(j == 0), stop=(j == last))
                if j == last:
                    pending_norm.append(u)

            def emit_norm(u):
                norms_done[u.i] += 2
                for s in range(2):
                    po = u.po[s]
                    # cast rowsum to f16, PE-broadcast the RAW rowsum to
                    # [64, BLK] (base partition 0), then reciprocal there
                    rinv = wp.tile([65, BLK], f16, tag="rinv", name="rinv")
                    nc.vector.tensor_copy(rinv[64:65, :], po[64:65, :])
                    pb = ps.tile([64, BLK], f32, tag="w", name="pb")
                    nc.tensor.matmul(pb[:], ones[64:65, 0:64],
                                     rinv[64:65, :], start=True, stop=True)
                    pbs = wp.tile([64, BLK], f32, tag="pbs", name="pbs")
                    nc.vector.reciprocal_approx_fast(pbs[:], pb[:])
                    if s == 0:
                        nc.vector.tensor_mul(u.ob[0:64, :], po[0:64, :], pbs[:])
                    else:
                        scr = wp.tile([64, BLK], f16, tag="scr", name="scr")
                        nc.vector.tensor_mul(scr[:], po[0:64, :], pbs[:])
                        nc.sync.dma_start(u.ob[64:128, :], scr[:])

            osb_live = {}

            def emit_wo(i, tl, h4, obufs, tag="w"):
                # one single-bank pass: hidden cols [h4*512:(h4+1)*512] of
                # out rows [tt*128:(tt+1)*128]; 4 passes complete a tile
                tt = i * 4 + tl
                pw = ps.tile([P, 512], f32, tag=tag, name="pw")
                for c in range(4):
                    nc.tensor.matmul(pw[:],
                                     obufs[c][:, tl * P:(tl + 1) * P],
                                     wo_all[:, c * H + h4 * 512:c * H + (h4 + 1) * 512],
                                     start=(c == 0), stop=(c == 3))
                if h4 == 0:
                    osb_live[tt] = wp.tile([P, H], f16, tag="osb", name="osb")
                osb = osb_live[tt]
                # evictions on the DVE while ACT is exp-saturated; alternate
                # only in the final drain (tag != "w") where ACT has slack
                if tag == "w" or h4 % 2 == 0:
                    nc.vector.tensor_copy(osb[:, h4 * 512:(h4 + 1) * 512], pw[:])
                else:
                    nc.scalar.activation(osb[:, h4 * 512:(h4 + 1) * 512],
                                         pw[:], FT.Copy)
                nc.sync.dma_start(
                    out_d[tt * P:(tt + 1) * P, h4 * 512:(h4 + 1) * 512],
                    osb[:, h4 * 512:(h4 + 1) * 512])
                if h4 == 3:
                    del osb_live[tt]

            flat = []      # (unit, j, block) S-jobs in emission order
            block_obufs = {}
            for i in range(NBLK):
                block_obufs[i] = {}
                for c in range(4):
                    ob = obp.tile([P, BLK], f16, tag="ob", name="ob")
                    block_obufs[i][c] = ob
                    u = Unit()
                    u.i, u.c = i, c
                    u.ob = ob
                    u.pts = {}
                    u.po = None
                    for j in range(4 * (i + 1)):
                        flat.append((u, j, i))

            queue = []     # PV jobs awaiting emission (depth pipeline)
            pending_norm = []
            norms_done = [0] * NBLK
            pending_wo = []
            pump_state = {"idx": 0, "cur_block": 0, "ctr": 0, "cur_u": None}

            def pump(n, allow_wo=True, max_block=NBLK - 1):
                done = 0
                while done < n and pump_state["idx"] < len(flat):
                    u, j, i = flat[pump_state["idx"]]
                    if i > max_block:
                        return
                    if i != pump_state["cur_block"]:
                        for tl in range(4):
                            for h4 in range(4):
                                pending_wo.append((pump_state["cur_block"], tl, h4))
                        pump_state["cur_block"] = i
                    if u is not pump_state["cur_u"]:
                        # unit boundary: flush old unit's PVs down to 1 (their
                        # exps are all done) so its po psum slots free well
                        # before this unit's first PV; the straggler pops
                        # after 2 of this unit's S-jobs (its exp then done)
                        while len(queue) > 1:
                            emit_PV(*queue.pop(0))
                        pump_state["cur_u"] = u
                        pump_state["fresh"] = 0
                    if u.po is None:
                        u.po = (pso.tile([65, BLK], f32, tag="o", name="poA"),
                                pso.tile([65, BLK], f32, tag="o", name="poB"))
                    # wo passes and ready PVs go into the PE queue BEFORE the
                    # S-pair: when S stalls waiting its psum slot (exp), the
                    # already-queued work keeps the PE streaming
                    pump_state["ctr"] += 1
                    if (allow_wo and pending_wo and pump_state["ctr"] % 2 == 0
                            and norms_done[pending_wo[0][0]] == 8):
                        wb, tl, h4 = pending_wo.pop(0)
                        emit_wo(wb, tl, h4, block_obufs[wb])
                    if queue and queue[0][0] is not u:
                        if pump_state["fresh"] >= 4:
                            emit_PV(*queue.pop(0))
                    else:
                        while len(queue) >= DEPTH:
                            emit_PV(*queue.pop(0))
                    emit_S(u, j)
                    queue.append((u, j))
                    pump_state["fresh"] += 1
                    # normalizes don't touch ACT: emit as soon as available
                    if pending_norm:
                        emit_norm(pending_norm.pop(0))
                    pump_state["idx"] += 1
                    done += 1

            # ---------- phase 1: projections + norm + rope + transpose ----------
            # QKV psums live in the "w" pool (1 bank each) so that both "a"
            # slots stay free for the S-jobs pumped into the phase-1 tail.
            pending_tr = []
            for tt in range(NTT):
                if tt == 2:
                    # deferred wo load (see note at the input DMAs above)
                    nc.gpsimd.dma_start(
                        wo_all[:].rearrange("p (c h) -> p c h", h=H),
                        wo_d[:].rearrange("(c p) h -> p c h", p=P))
                # before pumping starts (tiles 0-4) there is no filler work
                # between consecutive tiles' QKV chains: alternate the psum
                # ring per tile so tile t+1 never waits on tile t's drain
                qtag = "w" if (tt >= 5 or tt % 2) else "a"
                pa_q = ps.tile([P, 512], f32, tag=qtag, name="pa_q")
                pa_kv = ps.tile([P, 512], f32, tag=qtag, name="pa_kv")
                for hc in range(NHC):
                    lhs = xt_all[:, tt * H + hc * P:tt * H + (hc + 1) * P]
                    wof = hc * 768
                    nc.tensor.matmul(pa_q[:], lhs,
                                     wqkv_all[:, wof:wof + 512],
                                     start=(hc == 0), stop=(hc == NHC - 1),
                                     skip_group_check=True)
                    nc.tensor.matmul(pa_kv[:, 0:256], lhs,
                                     wqkv_all[:, wof + 512:wof + 768],
                                     start=(hc == 0), stop=(hc == NHC - 1),
                                     skip_group_check=True)
                # v eviction (no norm): one strided copy into both kv slots
                vt = vsb[tt]
                nc.vector.tensor_copy(
                    vt[:, 0:130].rearrange("p (s c) -> p s c", c=65)[:, :, 0:64],
                    pa_kv[:, 128:256].rearrange("p (s c) -> p s c", c=64))
                # q+k rmsnorm: squares on ACT, one fused reduce/Ln/Exp for all
                # 10 head-groups (8 q + 2 k)
                sq = wp.tile([P, DQ], f32, tag="sq")
                nc.scalar.activation(sq[:], pa_q[:], FT.Square)
                ksq = wp.tile([P, 128], f32, tag="ksq")
                nc.scalar.activation(ksq[:], pa_kv[:, 0:128], FT.Square)
                red = wp.tile([P, 10], f32, tag="red")
                nc.vector.reduce_sum(red[:, 0:8].unsqueeze(-1),
                                     sq[:].rearrange("p (h d) -> p h d", d=D), axis=AX.X)
                nc.vector.reduce_sum(red[:, 8:10].unsqueeze(-1),
                                     ksq[:].rearrange("p (h d) -> p h d", d=D), axis=AX.X)
                srt = wp.tile([P, 10], f32, tag="srt")
                nc.scalar.activation(srt[:], red[:], FT.Ln, scale=1.0 / D, bias=epsb[:])
                rstd = wp.tile([P, 10], f32, tag="rstd")
                nc.scalar.activation(rstd[:], srt[:], FT.Exp, scale=-0.5)
                qn = wp.tile([P, DQ], f16, tag="qn")
                qn3 = qn[:].rearrange("p (h d) -> p h d", d=D)
                nc.vector.tensor_mul(qn3, pa_q[:].rearrange("p (h d) -> p h d", d=D),
                                     rstd[:, 0:8].unsqueeze(-1).broadcast_to([P, NQ, D]))
                kn = wp.tile([P, 128], f16, tag="kn")
                kn3 = kn[:].rearrange("p (h d) -> p h d", d=D)
                nc.vector.tensor_mul(kn3, pa_kv[:, 0:128].rearrange("p (h d) -> p h d", d=D),
                                     rstd[:, 8:10].unsqueeze(-1).broadcast_to([P, 2, D]))
                # rope: cos-products on GPSIMD run in parallel with the
                # sin-products on DVE, halving the chain latency that gates
                # the PE transposes
                cosq = ropeq_sb[:, tt * 128:tt * 128 + 64]
                sinq = ropeq_sb[:, tt * 128 + 64:tt * 128 + 128]
                tcos = wp.tile([P, DQ], f16, tag="tcos")
                nc.gpsimd.tensor_mul(tcos[:].rearrange("p (h d) -> p h d", d=D), qn3,
                                     cosq.unsqueeze(1).broadcast_to([P, NQ, D]))
                rp = wp.tile([P, DQ], f16, tag="rp")
                rp3 = rp[:].rearrange("p (h d) -> p h d", d=D)
                nc.gpsimd.tensor_mul(rp3[:, :, 0:32], qn3[:, :, 32:64],
                                     sinq[:, 0:32].unsqueeze(1).broadcast_to([P, NQ, 32]))
                nc.gpsimd.tensor_mul(rp3[:, :, 32:64], qn3[:, :, 0:32],
                                     sinq[:, 32:64].unsqueeze(1).broadcast_to([P, NQ, 32]))
                qrope = wp.tile([P, DQ], f16, tag="qrope")
                nc.gpsimd.tensor_add(qrope[:], tcos[:], rp[:])
                # rope k
                cosk = ropek_sb[:, tt * 128:tt * 128 + 64]
                sink = ropek_sb[:, tt * 128 + 64:tt * 128 + 128]
                ktcos = wp.tile([P, 128], f16, tag="ktcos")
                nc.gpsimd.tensor_mul(ktcos[:].rearrange("p (h d) -> p h d", d=D), kn3,
                                     cosk.unsqueeze(1).broadcast_to([P, 2, D]))
                krp = wp.tile([P, 128], f16, tag="krp")
                krp3 = krp[:].rearrange("p (h d) -> p h d", d=D)
                nc.gpsimd.tensor_mul(krp3[:, :, 0:32], kn3[:, :, 32:64],
                                     sink[:, 0:32].unsqueeze(1).broadcast_to([P, 2, 32]))
                nc.gpsimd.tensor_mul(krp3[:, :, 32:64], kn3[:, :, 0:32],
                                     sink[:, 32:64].unsqueeze(1).broadcast_to([P, 2, 32]))
                krope = wp.tile([P, 128], f16, tag="krope")
                nc.gpsimd.tensor_add(krope[:], ktcos[:], krp[:])

                # transposes to [d, T]: early tiles (needed promptly by the
                # first pump jobs) go through the PE; later tiles use the DMA
                # xbar transpose engine (no PE work, no psum-ring contention,
                # no DVE evictions — its extra latency is hidden by the tile
                # cadence). Deferred one tile either way.
                def mk_transp(tt, qrope, krope):
                    def emit_pe():
                        ptr = ps.tile([P, 512], f16, tag="a", name="ptr")
                        for c in range(4):
                            nc.tensor.transpose(ptr[:, c * P:(c + 1) * P],
                                                qrope[:, c * P:(c + 1) * P], ident[:])
                        nc.vector.tensor_copy(qT3[:, :, tt * P:(tt + 1) * P],
                                              ptr[:].rearrange("p (c t) -> p c t", t=P))
                        ptk = ps.tile([P, P], f16, tag="a", name="ptk")
                        nc.tensor.transpose(ptk[:], krope[:], ident[:])
                        nc.vector.tensor_copy(kT[:, tt * P:(tt + 1) * P], ptk[:])

                    def emit_dma():
                        for c in range(4):
                            nc.sync.dma_start(qT3[:, c, tt * P:(tt + 1) * P],
                                              qrope[:, c * P:(c + 1) * P],
                                              transpose=True)
                        nc.sync.dma_start(kT[:, tt * P:(tt + 1) * P],
                                          krope[:], transpose=True)
                    return emit_pe if tt < 5 else emit_dma
                # PE transposes (tiles 0-4: "a" ring is pump-free there)
                # stay deferred one tile; DMA transposes (tiles 5+) emit
                # immediately — the SP queue is idle mid-phase-1, so the
                # rope-gated trigger wait blocks nothing, and kT/qT land a
                # tile earlier for the pump
                if tt < 5:
                    pending_tr.append(mk_transp(tt, qrope, krope))
                    if len(pending_tr) > 1:
                        pending_tr.pop(0)()
                else:
                    for fn in pending_tr:
                        fn()
                    pending_tr = []
                    mk_transp(tt, qrope, krope)()
                if tt >= 5:
                    # qT/kT/v complete through tile tt-1 -> blocks up to
                    # (tt-4)//4 are safe to pump; no wo during phase 1 (its
                    # psum pool is used by the QKV accumulations). Quotas
                    # sized to keep ACT (exp) ~85% fed alongside the QKV
                    # stream without outrunning block availability.
                    quota = 4 if tt < 9 else (8 if tt < 13 else 16)
                    pump(quota, allow_wo=False, max_block=max(0, (tt - 4) // 4))
            for fn in pending_tr:
                fn()

            # ---------- phase 2/3 remainder ----------
            pump(len(flat))
            while queue:
                emit_PV(*queue.pop(0))
            for u in pending_norm:
                emit_norm(u)
            pending_norm = []
            for tl in range(4):
                for h4 in range(4):
                    pending_wo.append((NBLK - 1, tl, h4))
            for k, (wb, tl, h4) in enumerate(pending_wo):
                emit_wo(wb, tl, h4, block_obufs[wb],
                        tag=("a" if k % 2 else "w"))

    nc.compile()
    return nc


def _host_inputs(x, Wq, Wk, Wv, Wo, q_ln_w, k_ln_w):
    x = np.asarray(x, np.float32)
    Wq = np.asarray(Wq, np.float32)
    Wk = np.asarray(Wk, np.float32)
    Wv = np.asarray(Wv, np.float32)
    Wo = np.asarray(Wo, np.float32)
    q_ln_w = np.asarray(q_ln_w, np.float64)
    k_ln_w = np.asarray(k_ln_w, np.float64)

    inv_freq = 1.0 / (1e6 ** (np.arange(0, D, 2, dtype=np.float64) / D))
    t = np.arange(T, dtype=np.float64)
    freqs = np.outer(t, inv_freq)
    emb = np.concatenate([freqs, freqs], -1)
    cos, sin = np.cos(emb), np.sin(emb)
    rot = (np.arange(D) + 32) % D
    sign = np.where(np.arange(D) < 32, -1.0, 1.0)

    def rope_tab(w):
        cw = w[None, :] * cos
        sw = sign[None, :] * w[rot][None, :] * sin
        tab = np.concatenate([cw, sw], -1).astype(np.float16)
        # [tt*128+p, c] -> [p*16+tt, c] (4KB DMA lines per partition)
        return np.ascontiguousarray(
            tab.reshape(NTT, P, 128).transpose(1, 0, 2).reshape(T, 128))

    ropeq = rope_tab(q_ln_w)
    ropek = rope_tab(k_ln_w)
    pp_, gg_ = np.meshgrid(np.arange(P), np.arange(P), indexing="ij")
    negI = np.where(pp_ == gg_, -30000.0, 0.0)
    ustr = (pp_ > gg_).astype(np.float64)
    mtab = np.concatenate([negI, ustr], axis=1).astype(np.float16)

    in_maps = []
    xTs = []
    for b in range(2):
        # [tt*128+p, hc*128+c] = x[b][tt*128+c, hc*128+p]: per-(p, tile)
        # 4KB-contiguous DMA lines
        A = x[b].astype(np.float16).reshape(NTT, P, NHC, P)  # [tt, c, hc, p]
        xTs.append(np.ascontiguousarray(
            A.transpose(0, 3, 2, 1).reshape(T, H)))
    for core in range(8):
        b, g = core // 4, core % 4
        xT = xTs[b]
        heads = []
        for c in range(4):
            heads += [g * 8 + c, g * 8 + c + 4]
        wqkv = np.concatenate(
            [Wq[:, h * D:(h + 1) * D] for h in heads]
            + [Wk[:, g * 128:(g + 1) * 128], Wv[:, g * 128:(g + 1) * 128]],
            axis=1).astype(np.float16)
        # [hc*128+p, c] -> [p*16+hc, c] (long per-partition DMA lines)
        wqkv = np.ascontiguousarray(
            wqkv.reshape(NHC, P, 768).transpose(1, 0, 2).reshape(H, 768))
        wo = np.ascontiguousarray(
            np.concatenate([Wo[h * D:(h + 1) * D, :] for h in heads], axis=0)
        ).astype(np.float16)
        in_maps.append({
            "xT": xT, "wqkv": wqkv, "wo": wo,
            "ropeq": ropeq, "ropek": ropek, "mtab": mtab,
        })
    return in_maps


def get_program():
    if "nc" not in _CACHE:
        _CACHE["nc"] = _build_program()
    return _CACHE["nc"]


def run(inputs, trace=False, tmpdir=None):
    nc = get_program()
    in_maps = _host_inputs(**inputs)
    res = run_bass_kernel_spmd(nc, in_maps, list(range(8)), trace=trace, tmpdir=tmpdir)
    out = np.zeros((2, T, H), np.float32)
    for core in range(8):
        out[core // 4] += res.results[core]["out"]
    return out, res


def kernel(**inputs) -> np.ndarray:
    out, _ = run(inputs, trace=False)
    return out
